# revision 24
# baseline (speedup 1.0000x reference)
"""Trainium2 Bass kernel for nn_CatConLayers (multi-head cross-attention over
time/category embeddings).

Sharding: 8 cores = 4 batches x 2 head-pairs. Each core computes, for its
batch b and heads {2g, 2g+1}:
  m_h   = WW_h^T @ q_in^T + Wk_h@bq_h   (WW_h = Wq_h @ Wk_h^T, host-fused)
  s_c^T = k_in^T-chunk-c @ [m_0|m_1]    (kT chunk stationary, heads batched)
  p~    = exp(s/sqrt(KQ))               (scores are tiny: no max-subtraction;
                                         the bk term cancels in the softmax)
  vo    = sum_c x_c^T @ p~_c            (value matmul, PSUM accumulation)
  Z     = ones^T @ p~ partial rows -> DVE strided reduce -> 1/Z columns
  out_h = (vo_h / Z_h) @ Wo_h           (normalization folded after Wo)
Host: builds k_in^T/q_in^T featurization (sinusoidal time embedding +
category-embedding rows; the ACT Sin table cannot be co-resident with the
Exp table, and on-device indirect-DMA gathers measured 1.1us each), fuses
Wq@Wk^T, shards inputs, sums the two head-pair partials per batch, adds bo.

Matmul operands on the scores path are bf16 (fp32 PSUM accumulation); the
value/output path dtype is selectable (fp32 default for accuracy).

The KQ dimension is permuted (sin block | cos block | emb0 | emb1) so the
interleaved sin/cos layout of the reference never has to be materialized
on-chip; Wq/Wk rows and q_in^T are permuted identically on host.
"""

import numpy as np
import ml_dtypes

import concourse.bass as bass
import concourse.mybir as mybir
import concourse.tile as tile
from concourse import bacc
from concourse.bass_utils import run_bass_kernel_spmd

# Problem shapes (hardcoded per harness contract)
N, T, H, KQ, LD, NREF, DT = 4, 1024, 4, 128, 128, 128, 64
NCORES = 8
TCH = T // 128  # 8 key chunks of 128

F32 = mybir.dt.float32
BF16 = mybir.dt.bfloat16
FP16 = mybir.dt.float16
AF = mybir.ActivationFunctionType

# matmul operand dtype scheme: "f16" = fp16 everywhere (1-pass matmuls,
# ~5e-4 absmax-rel), "hybrid" = bf16 scores + fp32 value (~6e-5, slower)
VALUE_DTYPE = "f16"

_CACHE = {}


def _build_program(vd_name):
    if vd_name == "f16":
        SD = VD = FP16
    elif vd_name == "bf16":
        SD = VD = BF16
    else:  # hybrid
        SD, VD = BF16, F32
    nc = bacc.Bacc("TRN2", target_bir_lowering=False, debug=False,
                   num_devices=NCORES)

    # inputs packed into three blobs, one per DMA queue:
    #   qblob: [qT | wq | wkT | bq2] (scalar engine; gates the first matmuls)
    #   kT: keys-transposed (sync engine)
    #   xblob: [x rearranged | wo] (gpsimd engine)
    kT_d = nc.dram_tensor("kT", [KQ, T], SD, kind="ExternalInput")
    qb_d = nc.dram_tensor("qblob", [128, 386], SD, kind="ExternalInput")
    xb_d = nc.dram_tensor("xblob", [128, T + 2 * LD], VD, kind="ExternalInput")
    out_d = nc.dram_tensor("out", [NREF, 2 * LD], F32, kind="ExternalOutput")

    inv_sqrt_kq = float(1.0 / np.sqrt(KQ))

    with tile.TileContext(nc) as tc:
        with tc.tile_pool(name="const", bufs=1) as cp, \
             tc.tile_pool(name="work", bufs=2) as sp, \
             tc.tile_pool(name="ps", bufs=2, space="PSUM") as pp:

            warm = cp.tile([128, 512], SD)
            nc.vector.memset(warm[:], 0.0)
            wps = pp.tile([128, 512], F32, tag="s1", bufs=4)
            for i in range(9):
                nc.tensor.matmul(out=wps[:], lhsT=warm[:, 0:128],
                                 rhs=warm[:], start=True, stop=True)

            ones_col = cp.tile([128, 1], VD)
            nc.vector.memset(ones_col[:], 1.0)
            one11 = cp.tile([1, 1], F32)
            nc.vector.memset(one11[:], 1.0)

            qblob = cp.tile([128, 386], SD)
            nc.scalar.dma_start(out=qblob[:], in_=qb_d[:])
            kT = cp.tile([KQ, T], SD)
            nc.sync.dma_start(out=kT[:], in_=kT_d[:])
            xblob = cp.tile([128, T + 2 * LD], VD)
            nc.gpsimd.dma_start(out=xblob[:], in_=xb_d[:])
            wkbq_sb = sp.tile([KQ, 2], F32, tag="bq", bufs=1)
            nc.vector.tensor_copy(out=wkbq_sb[:], in_=qblob[:, 384:386])

            # ---- m_h = WW_h^T @ q_in^T + Wk_h@bq_h, heads side by side,
            # with WW_h = Wq_h @ Wk_h^T and Wk_h@bq_h fused on host (pure
            # weight preprocessing). scores^T = k_in^T-chunks(stationary) @
            # [m_0|m_1]; the bk cross-term is constant over keys and cancels
            # exactly in the softmax.
            mp = pp.tile([128, 2 * NREF], F32, tag="s1", bufs=4)
            for h in range(2):
                nc.tensor.matmul(out=mp[:, h * 128:(h + 1) * 128],
                                 lhsT=qblob[:, h * 128:(h + 1) * 128],
                                 rhs=qblob[:, 256:384], start=True, stop=True)
            ms = sp.tile([128, 2 * NREF], SD, tag="ms", bufs=1)
            for h in range(2):
                nc.vector.tensor_scalar_add(out=ms[:, h * 128:(h + 1) * 128],
                                            in0=mp[:, h * 128:(h + 1) * 128],
                                            scalar1=wkbq_sb[:, h:h + 1])

            # ---- scores^T + exp. p~^T layout: chunk c, head h at
            # pT_all[:, c*256 + h*128 ...] so value/Z matmuls batch heads.
            pT_all = cp.tile([128, 2 * T], VD)
            for c2 in range(TCH // 2):
                sc = pp.tile([128, 512], F32, tag="s1", bufs=4)
                for j in range(2):
                    c = c2 * 2 + j
                    nc.tensor.matmul(out=sc[:, j * 256:(j + 1) * 256],
                                     lhsT=kT[:, c * 128:(c + 1) * 128],
                                     rhs=ms[:], start=True, stop=True)
                nc.scalar.activation(
                    out=pT_all[:, c2 * 512:(c2 + 1) * 512],
                    in_=sc[:], func=AF.Exp, scale=inv_sqrt_kq)

            # ---- softmax denominators: partial Z rows via ones^T @ p~T
            # (4 matmuls of N=512), reduced on DVE, transposed to columns
            # by tiny matmuls, then 1/Z.
            zrow = pp.tile([1, 2 * NREF], F32, tag="s1", bufs=4)
            for c in range(TCH):
                nc.tensor.matmul(out=zrow[:],
                                 lhsT=ones_col[:],
                                 rhs=pT_all[:, c * 256:(c + 1) * 256],
                                 start=(c == 0), stop=(c == TCH - 1))
            zr_sb = sp.tile([1, 2 * NREF], F32, tag="zrs", bufs=1)
            nc.vector.tensor_copy(out=zr_sb[:], in_=zrow[:])
            rinv = []
            for h in range(2):
                zc_ps = pp.tile([NREF, 1], F32, tag="s1", bufs=4)
                nc.tensor.matmul(out=zc_ps[:],
                                 lhsT=zr_sb[:, h * 128:(h + 1) * 128],
                                 rhs=one11[:], start=True, stop=True)
                ri = sp.tile([NREF, 1], F32, tag="ri", bufs=2)
                nc.vector.reciprocal(out=ri[:], in_=zc_ps[:])
                rinv.append(ri)

            # ---- value matmul: vo[v, c-interleaved q pairs] accumulated
            # over the 8 key chunks; both heads per matmul.
            vo = pp.tile([128, 2 * NREF], F32, tag="w2", bufs=2)
            for c in range(TCH):
                nc.tensor.matmul(out=vo[:],
                                 lhsT=xblob[:, c * 128:(c + 1) * 128],
                                 rhs=pT_all[:, c * 256:(c + 1) * 256],
                                 start=(c == 0), stop=(c == TCH - 1))

            # ---- output projection per head, then normalize+combine
            ot = sp.tile([128, 2 * NREF], VD, tag="ots", bufs=1)
            nc.vector.tensor_copy(out=ot[:, 0:128], in_=vo[:, 0:128])
            nc.scalar.copy(out=ot[:, 128:256], in_=vo[:, 128:256])
            fin = pp.tile([NREF, 2 * LD], F32, tag="s1", bufs=4)
            for h in range(2):
                nc.tensor.matmul(out=fin[:, h * 128:(h + 1) * 128],
                                 lhsT=ot[:, h * 128:(h + 1) * 128],
                                 rhs=xblob[:, T + h * LD:T + (h + 1) * LD],
                                 start=True, stop=True)
            res = sp.tile([NREF, 2 * LD], F32, tag="res", bufs=1)
            nc.vector.tensor_scalar_mul(out=res[:, 0:128], in0=fin[:, 0:128],
                                        scalar1=rinv[0][:, :1])
            nc.scalar.activation(out=res[:, 128:256], in_=fin[:, 128:256],
                                 func=AF.Copy, scale=rinv[1][:, :1])
            nc.sync.dma_start(out=out_d[:], in_=res[:])

    nc.compile()
    return nc


def _get_program(vd_name=None):
    vd_name = vd_name or VALUE_DTYPE
    if vd_name not in _CACHE:
        _CACHE[vd_name] = _build_program(vd_name)
    return _CACHE[vd_name]


def _host_prep(ts, ys0, ys1, emb0, emb1):
    """Full k_in^T (permuted) per batch and q_in^T."""
    div = np.exp(np.arange(0, DT, 2, dtype=np.float32)
                 * (-np.log(10.0) / DT)).astype(np.float32)  # (32,)
    ang = 48.0 * ts[:, :, None].astype(np.float32) * div[None, None, :]
    kT = np.empty((N, KQ, T), np.float32)
    kT[:, 0:32] = np.sin(ang).transpose(0, 2, 1)
    kT[:, 32:64] = np.cos(ang).transpose(0, 2, 1)
    kT[:, 64:96] = emb0[ys0].transpose(0, 2, 1)
    kT[:, 96:128] = emb1[ys1].transpose(0, 2, 1)

    ref = np.linspace(0.0, 1.0, NREF, dtype=np.float32)
    ang_r = 48.0 * ref[:, None] * div[None, :]  # (NREF, 32)
    qT = np.empty((KQ, NREF), np.float32)
    qT[0:32] = np.sin(ang_r).T
    qT[32:64] = np.cos(ang_r).T
    qT[64:96] = emb0[100][:, None]
    qT[96:128] = emb1[50][:, None]
    return kT, qT


def _make_in_maps(ts, ys0, ys1, x, emb0, emb1, Wq, bq, Wk, bk, Wo, vd_name):
    if vd_name == "f16":
        sd = vd = np.float16
    elif vd_name == "bf16":
        sd = vd = ml_dtypes.bfloat16
    else:  # hybrid
        sd, vd = ml_dtypes.bfloat16, np.float32
    bf = sd
    ts = np.asarray(ts, np.float32)
    x = np.asarray(x, np.float32)
    emb0 = np.asarray(emb0, np.float32)
    emb1 = np.asarray(emb1, np.float32)
    ys0 = np.asarray(ys0).astype(np.int64)
    ys1 = np.asarray(ys1).astype(np.int64)

    kT, qT = _host_prep(ts, ys0, ys1, emb0, emb1)
    # KQ permutation: (sin block | cos block | emb0 | emb1) -> reference order
    perm = np.concatenate([2 * np.arange(32), 2 * np.arange(32) + 1,
                           64 + np.arange(32), 96 + np.arange(32)])
    Wq_p = np.asarray(Wq, np.float32)[perm]
    Wk_p = np.asarray(Wk, np.float32)[perm]
    bq2 = np.asarray(bq, np.float32).reshape(H, KQ)
    bk2 = np.asarray(bk, np.float32).reshape(H, KQ)
    Wo = np.asarray(Wo, np.float32)
    # x rearranged: chunk c on cols [c*128,(c+1)*128), key t=c*128+p on part p
    xr = np.ascontiguousarray(
        x.reshape(N, TCH, 128, LD).transpose(0, 2, 1, 3).reshape(N, 128, T))

    in_maps = []
    for c in range(NCORES):
        b, hg = c // 2, c % 2
        # wo laid out (LD, 2*LD): local head h rows at cols [h*LD,(h+1)*LD)
        wo2 = np.ascontiguousarray(
            Wo[hg * 256:(hg + 1) * 256, :].reshape(2, LD, LD)
            .transpose(1, 0, 2).reshape(LD, 2 * LD))
        ww = np.concatenate(
            [Wq_p[:, (2 * hg + h) * 128:(2 * hg + h + 1) * 128]
             @ Wk_p[:, (2 * hg + h) * 128:(2 * hg + h + 1) * 128].T
             for h in range(2)], axis=1)  # (KQ, 2*KQ): WW_h[e, c]
        wkbq = np.stack(
            [Wk_p[:, (2 * hg + h) * 128:(2 * hg + h + 1) * 128]
             @ bq2[2 * hg + h] for h in range(2)], axis=1)  # (KQ, 2)
        qblob = np.concatenate([ww, qT, wkbq], axis=1)
        xblob = np.concatenate([xr[b], wo2], axis=1)
        in_maps.append(dict(
            kT=kT[b].astype(bf),
            qblob=np.ascontiguousarray(qblob).astype(bf),
            xblob=np.ascontiguousarray(xblob).astype(vd),
        ))
    return in_maps


def kernel(ts, ys0, ys1, x, emb0, emb1, Wq, bq, Wk, bk, Wo, bo):
    in_maps = _make_in_maps(ts, ys0, ys1, x, emb0, emb1, Wq, bq, Wk, bk, Wo,
                            VALUE_DTYPE)
    nc = _get_program()
    res = run_bass_kernel_spmd(nc, in_maps, list(range(NCORES)))
    bo = np.asarray(bo, np.float32)
    out = np.empty((N, NREF, LD), np.float32)
    for b in range(N):
        r0, r1 = res.results[2 * b]["out"], res.results[2 * b + 1]["out"]
        out[b] = (r0[:, :LD] + r0[:, LD:] + r1[:, :LD] + r1[:, LD:]
                  + bo[None, :])
    return out


# revision 25
# speedup vs baseline: 1.0685x; 1.0685x over previous
"""Trainium2 Bass kernel for nn_CatConLayers (multi-head cross-attention over
time/category embeddings).

Sharding: 8 cores = 4 batches x 2 head-pairs. Each core computes, for its
batch b and heads {2g, 2g+1}:
  m_h   = WW_h^T @ q_in^T + Wk_h@bq_h   (WW_h = Wq_h @ Wk_h^T, host-fused)
  s_c^T = k_in^T-chunk-c @ [m_0|m_1]    (kT chunk stationary, heads batched)
  p~    = exp(s/sqrt(KQ))               (scores are tiny: no max-subtraction;
                                         the bk term cancels in the softmax)
  vo    = sum_c x_c^T @ p~_c            (value matmul, PSUM accumulation)
  Z     = ones^T @ p~ partial rows -> DVE strided reduce -> 1/Z columns
  out_h = (vo_h / Z_h) @ Wo_h           (normalization folded after Wo)
Host: builds k_in^T/q_in^T featurization (sinusoidal time embedding +
category-embedding rows; the ACT Sin table cannot be co-resident with the
Exp table, and on-device indirect-DMA gathers measured 1.1us each), fuses
Wq@Wk^T, shards inputs, sums the two head-pair partials per batch, adds bo.

Matmul operands on the scores path are bf16 (fp32 PSUM accumulation); the
value/output path dtype is selectable (fp32 default for accuracy).

The KQ dimension is permuted (sin block | cos block | emb0 | emb1) so the
interleaved sin/cos layout of the reference never has to be materialized
on-chip; Wq/Wk rows and q_in^T are permuted identically on host.
"""

import numpy as np
import ml_dtypes

import concourse.bass as bass
import concourse.mybir as mybir
import concourse.tile as tile
from concourse import bacc
from concourse.bass_utils import run_bass_kernel_spmd

# Problem shapes (hardcoded per harness contract)
N, T, H, KQ, LD, NREF, DT = 4, 1024, 4, 128, 128, 128, 64
NCORES = 8
TCH = T // 128  # 8 key chunks of 128

F32 = mybir.dt.float32
BF16 = mybir.dt.bfloat16
FP16 = mybir.dt.float16
AF = mybir.ActivationFunctionType

# matmul operand dtype scheme: "f16" = fp16 everywhere (1-pass matmuls,
# ~5e-4 absmax-rel), "hybrid" = bf16 scores + fp32 value (~6e-5, slower)
VALUE_DTYPE = "f16"

_CACHE = {}


def _build_program(vd_name):
    if vd_name == "f16":
        SD = VD = FP16
    elif vd_name == "bf16":
        SD = VD = BF16
    else:  # hybrid
        SD, VD = BF16, F32
    nc = bacc.Bacc("TRN2", target_bir_lowering=False, debug=False,
                   num_devices=NCORES)

    # inputs packed into three blobs, one per DMA queue:
    #   qblob: [qT | wq | wkT | bq2] (scalar engine; gates the first matmuls)
    #   kT: keys-transposed (sync engine)
    #   xblob: [x rearranged | wo] (gpsimd engine)
    kT_d = nc.dram_tensor("kT", [KQ, T], SD, kind="ExternalInput")
    qb_d = nc.dram_tensor("qblob", [128, 386], SD, kind="ExternalInput")
    xb_d = nc.dram_tensor("xblob", [128, T + 2 * LD], VD, kind="ExternalInput")
    out_d = nc.dram_tensor("out", [NREF, 2 * LD], F32, kind="ExternalOutput")

    inv_sqrt_kq = float(1.0 / np.sqrt(KQ))

    with tile.TileContext(nc) as tc:
        with tc.tile_pool(name="const", bufs=1) as cp, \
             tc.tile_pool(name="work", bufs=2) as sp, \
             tc.tile_pool(name="ps", bufs=2, space="PSUM") as pp:

            warm = cp.tile([128, 512], SD)
            nc.vector.memset(warm[:], 0.0)
            wps = pp.tile([128, 512], F32, tag="s1", bufs=3)
            for i in range(9):
                nc.tensor.matmul(out=wps[:], lhsT=warm[:, 0:128],
                                 rhs=warm[:], start=True, stop=True)

            ones_col = cp.tile([128, 1], VD)
            nc.vector.memset(ones_col[:], 1.0)
            one11 = cp.tile([1, 1], F32)
            nc.vector.memset(one11[:], 1.0)

            qblob = cp.tile([128, 384], SD)
            nc.scalar.dma_start(out=qblob[:], in_=qb_d[:, 0:384])
            wkbq16 = cp.tile([KQ, 2], SD)
            nc.sync.dma_start(out=wkbq16[:], in_=qb_d[:, 384:386])
            kT = cp.tile([KQ, T], SD)
            nc.sync.dma_start(out=kT[:], in_=kT_d[:])
            xblob = cp.tile([128, T + 2 * LD], VD)
            nc.gpsimd.dma_start(out=xblob[:], in_=xb_d[:])
            wkbq_sb = sp.tile([KQ, 2], F32, tag="bq", bufs=1)
            nc.vector.tensor_copy(out=wkbq_sb[:], in_=wkbq16[:])

            # ---- m_h = WW_h^T @ q_in^T + Wk_h@bq_h, heads side by side,
            # with WW_h = Wq_h @ Wk_h^T and Wk_h@bq_h fused on host (pure
            # weight preprocessing). scores^T = k_in^T-chunks(stationary) @
            # [m_0|m_1]; the bk cross-term is constant over keys and cancels
            # exactly in the softmax.
            mp = pp.tile([128, 2 * NREF], F32, tag="s1", bufs=3)
            for h in range(2):
                nc.tensor.matmul(out=mp[:, h * 128:(h + 1) * 128],
                                 lhsT=qblob[:, h * 128:(h + 1) * 128],
                                 rhs=qblob[:, 256:384], start=True, stop=True)
            ms = sp.tile([128, 2 * NREF], SD, tag="ms", bufs=1)
            nc.vector.tensor_scalar_add(out=ms[:, 0:128], in0=mp[:, 0:128],
                                        scalar1=wkbq_sb[:, 0:1])
            nc.scalar.activation(out=ms[:, 128:256], in_=mp[:, 128:256],
                                 func=AF.Identity, bias=wkbq_sb[:, 1:2],
                                 scale=1.0)

            # ---- scores^T + exp. p~^T layout: chunk c, head h at
            # pT_all[:, c*256 + h*128 ...] so value/Z matmuls batch heads.
            pT_all = cp.tile([128, 2 * T], VD)
            for c4 in range(TCH // 4):
                sc = pp.tile([128, 1024], F32, tag="sc", bufs=2)
                for j in range(4):
                    c = c4 * 4 + j
                    nc.tensor.matmul(out=sc[:, j * 256:(j + 1) * 256],
                                     lhsT=kT[:, c * 128:(c + 1) * 128],
                                     rhs=ms[:], start=True, stop=True)
                nc.scalar.activation(
                    out=pT_all[:, c4 * 1024:(c4 + 1) * 1024],
                    in_=sc[:], func=AF.Exp, scale=inv_sqrt_kq)

            # ---- softmax denominators: partial Z rows via ones^T @ p~T
            # (4 matmuls of N=512), reduced on DVE, transposed to columns
            # by tiny matmuls, then 1/Z.
            zrow = pp.tile([1, 2 * NREF], F32, tag="s1", bufs=3)
            for c in range(TCH):
                nc.tensor.matmul(out=zrow[:],
                                 lhsT=ones_col[:],
                                 rhs=pT_all[:, c * 256:(c + 1) * 256],
                                 start=(c == 0), stop=(c == TCH - 1))
            zr_sb = sp.tile([1, 2 * NREF], F32, tag="zrs", bufs=1)
            nc.vector.tensor_copy(out=zr_sb[:], in_=zrow[:])
            rinv = []
            for h in range(2):
                zc_ps = pp.tile([NREF, 1], F32, tag="s1", bufs=3)
                nc.tensor.matmul(out=zc_ps[:],
                                 lhsT=zr_sb[:, h * 128:(h + 1) * 128],
                                 rhs=one11[:], start=True, stop=True)
                ri = sp.tile([NREF, 1], F32, tag="ri", bufs=2)
                nc.vector.reciprocal(out=ri[:], in_=zc_ps[:])
                rinv.append(ri)

            # ---- value matmul: vo[v, c-interleaved q pairs] accumulated
            # over the 8 key chunks; both heads per matmul.
            vo = pp.tile([128, 2 * NREF], F32, tag="w2", bufs=1)
            for c in range(TCH):
                nc.tensor.matmul(out=vo[:],
                                 lhsT=xblob[:, c * 128:(c + 1) * 128],
                                 rhs=pT_all[:, c * 256:(c + 1) * 256],
                                 start=(c == 0), stop=(c == TCH - 1))

            # ---- output projection per head, then normalize+combine
            ot = sp.tile([128, 2 * NREF], VD, tag="ots", bufs=1)
            nc.vector.tensor_copy(out=ot[:, 0:128], in_=vo[:, 0:128])
            nc.scalar.copy(out=ot[:, 128:256], in_=vo[:, 128:256])
            fin = pp.tile([NREF, 2 * LD], F32, tag="s1", bufs=3)
            for h in range(2):
                nc.tensor.matmul(out=fin[:, h * 128:(h + 1) * 128],
                                 lhsT=ot[:, h * 128:(h + 1) * 128],
                                 rhs=xblob[:, T + h * LD:T + (h + 1) * LD],
                                 start=True, stop=True)
            res = sp.tile([NREF, 2 * LD], F32, tag="res", bufs=1)
            nc.vector.tensor_scalar_mul(out=res[:, 0:128], in0=fin[:, 0:128],
                                        scalar1=rinv[0][:, :1])
            nc.scalar.activation(out=res[:, 128:256], in_=fin[:, 128:256],
                                 func=AF.Copy, scale=rinv[1][:, :1])
            nc.sync.dma_start(out=out_d[:], in_=res[:])

    nc.compile()
    return nc


def _get_program(vd_name=None):
    vd_name = vd_name or VALUE_DTYPE
    if vd_name not in _CACHE:
        _CACHE[vd_name] = _build_program(vd_name)
    return _CACHE[vd_name]


def _host_prep(ts, ys0, ys1, emb0, emb1):
    """Full k_in^T (permuted) per batch and q_in^T."""
    div = np.exp(np.arange(0, DT, 2, dtype=np.float32)
                 * (-np.log(10.0) / DT)).astype(np.float32)  # (32,)
    ang = 48.0 * ts[:, :, None].astype(np.float32) * div[None, None, :]
    kT = np.empty((N, KQ, T), np.float32)
    kT[:, 0:32] = np.sin(ang).transpose(0, 2, 1)
    kT[:, 32:64] = np.cos(ang).transpose(0, 2, 1)
    kT[:, 64:96] = emb0[ys0].transpose(0, 2, 1)
    kT[:, 96:128] = emb1[ys1].transpose(0, 2, 1)

    ref = np.linspace(0.0, 1.0, NREF, dtype=np.float32)
    ang_r = 48.0 * ref[:, None] * div[None, :]  # (NREF, 32)
    qT = np.empty((KQ, NREF), np.float32)
    qT[0:32] = np.sin(ang_r).T
    qT[32:64] = np.cos(ang_r).T
    qT[64:96] = emb0[100][:, None]
    qT[96:128] = emb1[50][:, None]
    return kT, qT


def _make_in_maps(ts, ys0, ys1, x, emb0, emb1, Wq, bq, Wk, bk, Wo, vd_name):
    if vd_name == "f16":
        sd = vd = np.float16
    elif vd_name == "bf16":
        sd = vd = ml_dtypes.bfloat16
    else:  # hybrid
        sd, vd = ml_dtypes.bfloat16, np.float32
    bf = sd
    ts = np.asarray(ts, np.float32)
    x = np.asarray(x, np.float32)
    emb0 = np.asarray(emb0, np.float32)
    emb1 = np.asarray(emb1, np.float32)
    ys0 = np.asarray(ys0).astype(np.int64)
    ys1 = np.asarray(ys1).astype(np.int64)

    kT, qT = _host_prep(ts, ys0, ys1, emb0, emb1)
    # KQ permutation: (sin block | cos block | emb0 | emb1) -> reference order
    perm = np.concatenate([2 * np.arange(32), 2 * np.arange(32) + 1,
                           64 + np.arange(32), 96 + np.arange(32)])
    Wq_p = np.asarray(Wq, np.float32)[perm]
    Wk_p = np.asarray(Wk, np.float32)[perm]
    bq2 = np.asarray(bq, np.float32).reshape(H, KQ)
    bk2 = np.asarray(bk, np.float32).reshape(H, KQ)
    Wo = np.asarray(Wo, np.float32)
    # x rearranged: chunk c on cols [c*128,(c+1)*128), key t=c*128+p on part p
    xr = np.ascontiguousarray(
        x.reshape(N, TCH, 128, LD).transpose(0, 2, 1, 3).reshape(N, 128, T))

    in_maps = []
    for c in range(NCORES):
        b, hg = c // 2, c % 2
        # wo laid out (LD, 2*LD): local head h rows at cols [h*LD,(h+1)*LD)
        wo2 = np.ascontiguousarray(
            Wo[hg * 256:(hg + 1) * 256, :].reshape(2, LD, LD)
            .transpose(1, 0, 2).reshape(LD, 2 * LD))
        ww = np.concatenate(
            [Wq_p[:, (2 * hg + h) * 128:(2 * hg + h + 1) * 128]
             @ Wk_p[:, (2 * hg + h) * 128:(2 * hg + h + 1) * 128].T
             for h in range(2)], axis=1)  # (KQ, 2*KQ): WW_h[e, c]
        wkbq = np.stack(
            [Wk_p[:, (2 * hg + h) * 128:(2 * hg + h + 1) * 128]
             @ bq2[2 * hg + h] for h in range(2)], axis=1)  # (KQ, 2)
        qblob = np.concatenate([ww, qT, wkbq], axis=1)
        xblob = np.concatenate([xr[b], wo2], axis=1)
        in_maps.append(dict(
            kT=kT[b].astype(bf),
            qblob=np.ascontiguousarray(qblob).astype(bf),
            xblob=np.ascontiguousarray(xblob).astype(vd),
        ))
    return in_maps


def kernel(ts, ys0, ys1, x, emb0, emb1, Wq, bq, Wk, bk, Wo, bo):
    in_maps = _make_in_maps(ts, ys0, ys1, x, emb0, emb1, Wq, bq, Wk, bk, Wo,
                            VALUE_DTYPE)
    nc = _get_program()
    res = run_bass_kernel_spmd(nc, in_maps, list(range(NCORES)))
    bo = np.asarray(bo, np.float32)
    out = np.empty((N, NREF, LD), np.float32)
    for b in range(N):
        r0, r1 = res.results[2 * b]["out"], res.results[2 * b + 1]["out"]
        out[b] = (r0[:, :LD] + r0[:, LD:] + r1[:, :LD] + r1[:, LD:]
                  + bo[None, :])
    return out
